# revision 21
# baseline (speedup 1.0000x reference)
"""Trainium2 kernel for nn_DifferentiableBiquad.

Cascade of 4 biquad IIR filters over (B=32, L=524288), f32 in/out.

The pole radii are sigmoid(logit)*0.999 (actual inputs give r_max ~
0.71), so the cascade impulse response decays below 1e-4 of its peak
within ~25 lags. The IIR is computed as a truncated FIR, expressed as
banded block-Toeplitz matmuls on the TensorEngine in bf16 (the 2e-2
rel-err budget dwarfs bf16 quantization):

  - Per batch row, x is viewed as 128-sample blocks. Groups of 512
    blocks are DMA'd with 1KB runs (partition q holds 512 contiguous
    samples per group) and PE-transposed (4x 128x128 via the identity)
    into a per-row SBUF tile xt laid out in 4 phase slots
    [j3 | j0 | j1 | j2]: col (slot_j + g*128 + p) = block 4(g*128+p)+j
    transposed (partition = sample-within-block). One strided DVE/ACT
    copy per group evicts the 4 transposed phases from PSUM into their
    slot positions. The boundary block of group g (block "4p-1") is
    slot-j3 col g*128-1 == last block of group g-1 -- naturally
    adjacent, so no boundary copies; g=0 reads a memset zero col.
  - One [128, 512] f32 PSUM tile per group: five accumulating matmuls
    with stationary = 128 contiguous xt cols (block offsets -1,0,1,2,3
    in stride-4 block space) against the stacked tap matrix
    Hb = [H0T | H1T] produce y for the group. Overlapping write ranges
    of consecutive matmuls force program order; PSUM's has_written
    bits turn first touches into stores.
  - PSUM evictions (xt copies and y f32->bf16 casts) alternate between
    the Vector and Scalar engines; 4 groups batch into one output DMA
    (1KB runs) issued from the GpSimd (SWDGE) ring, inputs use the
    sync HWDGE ring, so no engine owns more than one copy per group.

Batch dim (32) is sharded over 8 NeuronCores (4 rows each); rows are
independent (zero initial state == one zero history block).
"""
import math

import numpy as np

NUM_FILTERS = 4
MAX_RADIUS = 0.999
B, L = 32, 524288
N_CORES = 8
ROWS_PER_CORE = B // N_CORES
NBLK = 128  # block size == SBUF partitions
GSZ = 4     # blocks per 512-sample output chunk


# ---------------------------------------------------------------- host math
def _coeffs_f32(log_radius, raw_angle):
    lr = np.asarray(log_radius, np.float32)
    ra = np.asarray(raw_angle, np.float32)
    radius = (np.float32(1.0) / (np.float32(1.0) + np.exp(-lr, dtype=np.float32))) * np.float32(MAX_RADIUS)
    angle = (np.float32(1.0) / (np.float32(1.0) + np.exp(-ra, dtype=np.float32))) * np.float32(math.pi)
    a1 = np.float32(-2.0) * radius * np.cos(angle, dtype=np.float32)
    a2 = radius * radius
    return a1.astype(np.float32), a2.astype(np.float32)


def _impulse_response(a1, a2, b0, b1, b2, T=256):
    h = np.zeros(T, np.float64)
    h[0] = 1.0
    for f in range(NUM_FILTERS):
        s1 = s2 = 0.0
        out = np.zeros(T, np.float64)
        for n in range(T):
            xn = h[n]
            yn = float(b0[f]) * xn + s1
            s1 = float(b1[f]) * xn - float(a1[f]) * yn + s2
            s2 = float(b2[f]) * xn - float(a2[f]) * yn
            out[n] = yn
        h = out
    return h


def _build_tap_matrices(inputs):
    a1, a2 = _coeffs_f32(inputs["log_radius"], inputs["raw_angle"])
    h = _impulse_response(
        a1, a2,
        np.asarray(inputs["b0"], np.float64),
        np.asarray(inputs["b1"], np.float64),
        np.asarray(inputs["b2"], np.float64),
    )
    hmax = np.abs(h).max()
    # 1e-4 relative truncation: tail l2 ~5e-5, far below the bf16
    # quantization noise (~2e-3) and the 2e-2 gate.
    tap_max = int(np.max(np.nonzero(np.abs(h) > 1e-4 * hmax)))
    assert tap_max <= 127, (
        f"impulse response too long for single-shift kernel (tap_max={tap_max})"
    )
    NC1 = max(1, min(128, tap_max))
    n_idx = np.arange(NBLK)
    m_idx = np.arange(NBLK)
    lag0 = n_idx[None, :] - m_idx[:, None]          # [m, n]
    H0T = np.where((lag0 >= 0) & (lag0 <= tap_max), h[np.clip(lag0, 0, 255)], 0.0)
    lag1 = 128 + n_idx[None, :NC1] - m_idx[:, None]  # [m, n]
    H1T = np.where((lag1 >= 1) & (lag1 <= tap_max), h[np.clip(lag1, 0, 255)], 0.0)
    return H0T.astype(np.float32), H1T.astype(np.float32)


def _build_hb(inputs):
    H0T, H1T = _build_tap_matrices(inputs)
    return np.concatenate([H0T, H1T], axis=1)  # [128, 128+NC1]


# ---------------------------------------------------------------- program
_PROGRAM_CACHE = {}


def build_program(n_rows, length, NC1):
    import concourse.mybir as mybir
    from concourse import bacc
    from concourse.tile import TileContext

    f32 = mybir.dt.float32
    bf16 = mybir.dt.bfloat16
    nblocks = length // NBLK
    nwin = nblocks // NBLK             # 128-chunk windows per row
    ngroups = nwin // GSZ              # 512-block groups per row
    assert nwin % GSZ == 0 and length % NBLK == 0
    W = GSZ * NBLK                     # 512 output samples per psum partition
    OSZ = 4                            # groups per output DMA
    assert ngroups % OSZ == 0
    SLOT = ngroups * NBLK              # cols per phase slot
    s3 = 32                            # slot j3 starts at col 32 (byte 64)
    XTC = s3 + GSZ * SLOT

    nc = bacc.Bacc("TRN2", target_bir_lowering=False, debug=False,
                   enable_asserts=False, num_devices=N_CORES)
    xin = nc.dram_tensor("xin", [n_rows, length], bf16, kind="ExternalInput")
    hb = nc.dram_tensor("hb", [NBLK, NBLK + NC1], bf16, kind="ExternalInput")
    ident = nc.dram_tensor("ident", [NBLK, NBLK], bf16, kind="ExternalInput")
    yout = nc.dram_tensor("yout", [n_rows, length], bf16, kind="ExternalOutput")

    with TileContext(nc) as tc:
        with (
            tc.tile_pool(name="const", bufs=1) as cpool,
            tc.tile_pool(name="vrow", bufs=6) as vpool,
            tc.tile_pool(name="xt", bufs=2) as xtpool,
            tc.tile_pool(name="stage", bufs=3) as spool,
            tc.tile_pool(name="pt", bufs=4, space="PSUM") as ptpool,
            tc.tile_pool(name="py", bufs=4, space="PSUM") as pypool,
        ):
            hb_sb = cpool.tile([NBLK, NBLK + NC1], bf16, tag="hb")
            nc.scalar.dma_start(out=hb_sb[:], in_=hb.ap())
            id_sb = cpool.tile([NBLK, NBLK], bf16, tag="id")
            nc.gpsimd.dma_start(out=id_sb[:], in_=ident.ap())

            # Input: per group, partition q holds 512 contiguous samples
            # (1KB bf16); transpose j recovers every-4th 128-block. Input
            # DMAs move pairs of groups to amortize fixed costs (first
            # group of row 0 loads alone so compute starts sooner).
            yout_v = yout.ap().rearrange(
                "r (H G p c) -> r H p G c", p=NBLK, G=OSZ, c=W
            )
            gsamp = GSZ * NBLK * NBLK
            for r in range(n_rows):
                in_chunks = [1, 1] + [2] * ((ngroups - 2) // 2) if r == 0 \
                    else [2] * (ngroups // 2)
                gof_chunk = []
                for ci, csz in enumerate(in_chunks):
                    g0 = sum(in_chunks[:ci])
                    gof_chunk += [(g0, k, csz) for k in range(csz)]

                xt = xtpool.tile([NBLK, XTC], bf16, tag="xt")
                nc.gpsimd.memset(xt[:, s3 - 1:s3], 0.0)
                # strided slot view: [p, slot, SLOT-col]
                xts = xt[:, s3:s3 + GSZ * SLOT].rearrange(
                    "p (s c) -> p s c", s=GSZ
                )
                vpair = None
                spair = None
                for g in range(ngroups):
                    g0, G_in, csz = gof_chunk[g]
                    if G_in == 0:
                        vpair = vpool.tile([NBLK, csz, GSZ, NBLK], bf16,
                                           tag="v")
                        src_ap = xin.ap()[r][
                            g0 * gsamp:(g0 + csz) * gsamp
                        ].rearrange(
                            "(G q j m) -> q G j m", G=csz, q=NBLK, j=GSZ,
                            m=NBLK,
                        )
                        nc.sync.dma_start(out=vpair[:], in_=src_ap)
                    v = vpair[:, G_in]
                    pt = ptpool.tile([NBLK, GSZ, NBLK], bf16, tag="pt")
                    for j in range(GSZ):
                        slot = (j + 1) % GSZ   # j3 lands in slot 0
                        nc.tensor.transpose(
                            pt[:, slot], v[:, j, :], id_sb[:],
                        )
                    # one strided copy scatters the 4 phases to slots;
                    # copies alternate engines so consecutive groups overlap
                    gb = g * NBLK
                    if g % 2 == 0:
                        nc.vector.tensor_copy(
                            out=xts[:, :, gb:gb + NBLK], in_=pt[:]
                        )
                    else:
                        nc.scalar.activation(
                            out=xts[:, :, gb:gb + NBLK], in_=pt[:],
                            func=mybir.ActivationFunctionType.Copy,
                        )
                    # banded matmuls: block offsets -1, 0, 1, 2, 3
                    py = pypool.tile([NBLK, W], f32, tag="py")
                    nc.tensor.matmul(
                        py[:, 0:NC1],
                        xt[:, s3 + gb - 1:s3 + gb - 1 + NBLK],
                        hb_sb[:, NBLK:NBLK + NC1],
                        start=True, stop=False, skip_group_check=True,
                    )
                    for dlt in range(GSZ):
                        lo = dlt * NBLK
                        hi = min(W, lo + NBLK + NC1)
                        off = s3 + ((dlt + 1) % GSZ) * SLOT + gb
                        nc.tensor.matmul(
                            py[:, lo:hi],
                            xt[:, off:off + NBLK],
                            hb_sb[:, 0:hi - lo],
                            start=False, stop=(dlt == GSZ - 1),
                            skip_group_check=True,
                        )
                    G_out = g % OSZ
                    if G_out == 0:
                        spair = spool.tile([NBLK, OSZ, W], bf16, tag="stage")
                    if g % 2 == 0:
                        nc.scalar.activation(
                            out=spair[:, G_out], in_=py[:],
                            func=mybir.ActivationFunctionType.Copy,
                        )
                    else:
                        nc.vector.tensor_copy(out=spair[:, G_out], in_=py[:])
                    last_batch = (r == n_rows - 1 and
                                  g // OSZ == ngroups // OSZ - 1)
                    if not last_batch:
                        if G_out == OSZ - 1:
                            nc.gpsimd.dma_start(
                                out=yout_v[r, g // OSZ], in_=spair[:]
                            )
                    else:
                        # drain the final batch incrementally across rings so
                        # the kernel does not end on one long DMA
                        H = g // OSZ
                        if G_out == 1:
                            nc.gpsimd.dma_start(
                                out=yout_v[r, H][:, 0:2], in_=spair[:, 0:2]
                            )
                        elif G_out == 2:
                            nc.scalar.dma_start(
                                out=yout_v[r, H][:, 2:3], in_=spair[:, 2:3]
                            )
                        elif G_out == 3:
                            nc.sync.dma_start(
                                out=yout_v[r, H][:, 3:4], in_=spair[:, 3:4]
                            )
    nc.compile()
    return nc


def _get_program(n_rows, length, NC1):
    key = (n_rows, length, NC1)
    if key not in _PROGRAM_CACHE:
        _PROGRAM_CACHE[key] = build_program(*key)
    return _PROGRAM_CACHE[key]


# ---------------------------------------------------------------- entry
def _run(inputs, trace=False):
    import ml_dtypes
    from concourse.bass_utils import run_bass_kernel_spmd

    bf16 = ml_dtypes.bfloat16
    x = np.ascontiguousarray(
        np.asarray(inputs["x"], np.float32).astype(bf16)
    )
    assert x.shape == (B, L)
    Hb = _build_hb(inputs).astype(bf16)
    NC1 = Hb.shape[1] - NBLK
    I = np.eye(NBLK, dtype=bf16)

    nc = _get_program(ROWS_PER_CORE, L, NC1)
    xs = x.reshape(N_CORES, ROWS_PER_CORE, L)
    in_maps = [
        {"xin": xs[c], "hb": Hb, "ident": I}
        for c in range(N_CORES)
    ]
    res = run_bass_kernel_spmd(nc, in_maps, core_ids=list(range(N_CORES)),
                               trace=trace)
    y = np.concatenate(
        [np.asarray(res.results[c]["yout"]).astype(np.float32)
         for c in range(N_CORES)],
        axis=0,
    ).reshape(B, L)
    return y, res


def kernel(x, log_radius, raw_angle, b0, b1, b2):
    y, _ = _run(dict(x=x, log_radius=log_radius, raw_angle=raw_angle,
                     b0=b0, b1=b1, b2=b2))
    return y
